# revision 7
# baseline (speedup 1.0000x reference)
"""Trainium2 Bass kernel for CrossEncoderMTL (weighted layer pooling + masked mean
+ section-routed adapter + reg/ord heads), data-parallel over batch across 8 cores.

Self-contained: hardcodes shapes; host-side numpy does sharding + index gathers;
the device kernel does all the heavy compute (streaming the 1 GiB hidden_states).
"""

import numpy as np

L, B, S, H, K = 4, 256, 256, 1024, 64
NCORES = 8
BL = B // NCORES          # 32 samples per core
NH = H // 128             # 8 h-chunks (h = 8*p + r mapping)
BPAIR = 2                 # samples per hidden_states DMA (2 MiB chunks)

_CACHE = {}
LAST_RESULT = None        # BassKernelResults of the most recent run (for profiling)


def _build_module(act_fn="Gelu"):
    from contextlib import ExitStack
    from concourse import bacc, mybir, tile

    f32 = mybir.dt.float32
    nc = bacc.Bacc(
        "TRN2", target_bir_lowering=False, debug=False, num_devices=NCORES
    )

    # ---- DRAM I/O (per-core shapes; device layouts precomputed on host) ----
    hs = nc.dram_tensor("hs", [L, BL, S, H], f32, kind="ExternalInput")
    # coeff[p, ((l*2+c)*BL + b)*BL + m] = softmax(w)[l]*mask[b,c*128+p]/msum[b]
    #                                     if m==b else 0
    coeff = nc.dram_tensor("coeff", [128, L * 2 * BL * BL], f32, kind="ExternalInput")
    # wd[p, (r*BL + b)*K + k] = W_down[sec[b], 8p+r, k]
    wd = nc.dram_tensor("wd", [128, NH * BL * K], f32, kind="ExternalInput")
    # wro[p, r*5+j] = [W_reg | W_ord][8p+r, j]
    wro = nc.dram_tensor("wro", [128, NH * 5], f32, kind="ExternalInput")
    # wuro[k, b*5+j] = (W_up[sec[b]] @ [W_reg|W_ord])[k, j]
    wuro = nc.dram_tensor("wuro", [K, BL * 5], f32, kind="ExternalInput")
    # dmask64[m, b*64+k] = 1.0 iff m == b   (diagonal extraction, down-proj)
    dmask64 = nc.dram_tensor("dmask64", [BL, BL * K], f32, kind="ExternalInput")
    # dmask5[m, b*5+j] = 1.0 iff m == b     (diagonal extraction, head A)
    dmask5 = nc.dram_tensor("dmask5", [BL, BL * 5], f32, kind="ExternalInput")
    # bd[b, k] = b_down[sec[b], k]
    bd = nc.dram_tensor("bd", [BL, K], f32, kind="ExternalInput")
    # bro[b, j] = (b_up[sec[b]] @ [W_reg|W_ord])[j] + [b_reg | b_ord][j]
    bro = nc.dram_tensor("bro", [BL, 5], f32, kind="ExternalInput")
    ident = nc.dram_tensor("ident", [BL, BL], f32, kind="ExternalInput")
    out = nc.dram_tensor("out", [BL, 5], f32, kind="ExternalOutput")

    with tile.TileContext(nc) as tc:
        with ExitStack() as ctx:
            consts = ctx.enter_context(tc.tile_pool(name="consts", bufs=1))
            hs_pool = ctx.enter_context(tc.tile_pool(name="hsp", bufs=4))
            work = ctx.enter_context(tc.tile_pool(name="work", bufs=1))

            # Constants go on the scalar-engine HWDGE ring so they never
            # head-of-line-block the hidden_states stream on the sync ring.
            def cload(dram, shape):
                t = consts.tile(shape, f32, tag=dram.name)
                nc.scalar.dma_start(t[:], dram.ap())
                return t

            id_sb = cload(ident, [BL, BL])
            wro_sb = cload(wro, [128, NH * 5])
            # Big constants are staggered into the pooling loop below so they
            # don't compete with the hidden_states stream at startup.
            coeff_sb = consts.tile([128, L * 2 * BL * BL], f32, tag="coeff")
            wd_sb = consts.tile([128, NH * BL * K], f32, tag="wd")
            wuro_sb = consts.tile([K, BL * 5], f32, tag="wuro")
            bd_sb = consts.tile([BL, K], f32, tag="bd")
            bro_sb = consts.tile([BL, 5], f32, tag="bro")
            dmask64_sb = consts.tile([BL, BL * K], f32, tag="dmask64")
            dmask5_sb = consts.tile([BL, BL * 5], f32, tag="dmask5")

            def load_coeff_blocks(ll):
                for c in range(2):
                    j = ll * 2 + c
                    blk = slice(j * BL * BL, (j + 1) * BL * BL)
                    nc.scalar.dma_start(coeff_sb[:, blk], coeff.ap()[:, blk])

            def load_wd_chunks(rs):
                for r in rs:
                    blk = slice(r * BL * K, (r + 1) * BL * K)
                    nc.scalar.dma_start(wd_sb[:, blk], wd.ap()[:, blk])

            load_coeff_blocks(0)

            # ---- PE warm-up: ~4us of tiny matmuls so HAM reaches K=8/8
            # before the real stream starts (runs during startup DMAs) ----
            with tc.tile_pool(name="pwarm", bufs=1, space="PSUM") as pwarm:
                wps = pwarm.tile([BL, BL], f32)
                for _ in range(48):
                    nc.tensor.matmul(wps[:], id_sb[:], id_sb[:],
                                     start=True, stop=True)

            # ---- pooling: feats[b, h] = sum_{l,s} w_l * c[b,s] * hs[l,b,s,h] ----
            # Zero-embedded coeff columns route sample b's contribution to psum
            # row b, so feats accumulates directly as [BL, H] with b on
            # partitions.
            feats_sb = work.tile([BL, H], f32)
            with tc.tile_pool(name="pf", bufs=1, space="PSUM") as pf_pool:
                pf = pf_pool.tile([BL, H], f32)
                hs_ap = hs.ap()
                idx = 0
                n_idx = L * BL * 2
                for l in range(L):
                    # Prefetch next layer's coeff block and other late constants
                    # on the scalar ring, one layer (~110us) ahead of use.
                    if l + 1 < L:
                        load_coeff_blocks(l + 1)
                    if l == 1:
                        load_wd_chunks(range(0, 4))
                    elif l == 2:
                        load_wd_chunks(range(4, NH))
                        nc.scalar.dma_start(wuro_sb[:], wuro.ap())
                        nc.scalar.dma_start(bd_sb[:], bd.ap())
                        nc.scalar.dma_start(bro_sb[:], bro.ap())
                    elif l == 3:
                        nc.scalar.dma_start(dmask64_sb[:], dmask64.ap())
                        nc.scalar.dma_start(dmask5_sb[:], dmask5.ap())
                    for bp in range(BL // BPAIR):
                        t = hs_pool.tile([128, BPAIR * 2 * 1024], f32, tag="hst")
                        src = hs_ap[l, bp * BPAIR:(bp + 1) * BPAIR].rearrange(
                            "b (c p) h -> p b c h", p=128
                        )
                        dst = t[:].rearrange("p (b c h) -> p b c h", b=BPAIR, c=2)
                        nc.sync.dma_start(dst, src)
                        for bb in range(BPAIR):
                            b = bp * BPAIR + bb
                            for c in range(2):
                                j = l * 2 + c
                                lhs = coeff_sb[
                                    :, (j * BL + b) * BL:(j * BL + b + 1) * BL]
                                for hh in range(2):
                                    base = bb * 2048 + c * 1024 + hh * 512
                                    nc.tensor.matmul(
                                        pf[:, hh * 512:(hh + 1) * 512],
                                        lhs,
                                        t[:, base:base + 512],
                                        start=(idx == 0),
                                        stop=(idx == n_idx - 1),
                                    )
                                idx += 1
                nc.vector.tensor_copy(feats_sb[:], pf[:])
            # pf's 2 PSUM banks are free again here.

            from concourse import mybir as _mb
            with tc.tile_pool(name="pt", bufs=2, space="PSUM") as pt_pool, \
                 tc.tile_pool(name="pd", bufs=1, space="PSUM") as pd_pool, \
                 tc.tile_pool(name="ph", bufs=1, space="PSUM") as ph_pool:
                # ---- transpose feats to h-on-partitions ----
                # featsT[:, r*BL + b][p] = feats[b, 8p + r]
                featsT = work.tile([128, NH * BL], f32)
                fview = feats_sb[:].rearrange("p (q r) -> p r q", r=NH)
                for r in range(NH):
                    pt = pt_pool.tile([128, BL], f32, tag="pt")
                    nc.tensor.transpose(pt[:], fview[:, r], id_sb[:])
                    nc.vector.tensor_copy(featsT[:, r * BL:(r + 1) * BL], pt[:])

                # ---- batched down-projection ----
                # bigD[m, b*K+k] = sum_h feats[m,h] * W_down[sec[b],h,k];
                # the diagonal m==b is the wanted z[b, k].
                bigD = pd_pool.tile([BL, BL * K], f32, tag="bigD")
                nmm = BL * K // 512                      # 4 matmuls of N=512
                for r in range(NH):
                    lhsT = featsT[:, r * BL:(r + 1) * BL]
                    for n in range(nmm):
                        nc.tensor.matmul(
                            bigD[:, n * 512:(n + 1) * 512],
                            lhsT,
                            wd_sb[:, r * BL * K + n * 512:
                                  r * BL * K + (n + 1) * 512],
                            start=(r == 0),
                            stop=(r == NH - 1),
                        )
                zmask = work.tile([BL, BL * K], f32)
                nc.vector.tensor_mul(zmask[:], bigD[:], dmask64_sb[:])
                z_bt = work.tile([BL, K], f32)
                nc.vector.tensor_reduce(
                    z_bt[:],
                    zmask[:].rearrange("p (g j) -> p j g", j=K),
                    _mb.AxisListType.X,
                    _mb.AluOpType.add,
                )
                zb = work.tile([BL, K], f32)
                nc.vector.tensor_add(zb[:], z_bt[:], bd_sb[:])
                h1_bt = work.tile([BL, K], f32)
                nc.scalar.activation(
                    h1_bt[:], zb[:], getattr(_mb.ActivationFunctionType, act_fn))
                # h1 with k on partitions for the head-A matmul
                pth = pt_pool.tile([K, BL], f32, tag="pt")
                nc.tensor.transpose(pth[:], h1_bt[:], id_sb[:])
                h1_sb = work.tile([K, BL], f32)
                nc.vector.tensor_copy(h1_sb[:], pth[:])

                # ---- heads ----
                # B-part: feats @ [W_reg|W_ord] -> pB[b, j]
                pB = ph_pool.tile([BL, 5], f32, tag="pB")
                for r in range(NH):
                    nc.tensor.matmul(
                        pB[:],
                        featsT[:, r * BL:(r + 1) * BL],
                        wro_sb[:, r * 5:(r + 1) * 5],
                        start=(r == 0),
                        stop=(r == NH - 1),
                    )
                # A-part: h1[b] @ (W_up[sec[b]] @ W_ro) for all (m, b) pairs,
                # then diagonal-extract.
                pA = ph_pool.tile([BL, BL * 5], f32, tag="pA")
                nc.tensor.matmul(pA[:], h1_sb[:], wuro_sb[:], start=True, stop=True)
                tmpA = work.tile([BL, BL * 5], f32)
                nc.vector.tensor_mul(tmpA[:], pA[:], dmask5_sb[:])
                redA = work.tile([BL, 5], f32)
                nc.vector.tensor_reduce(
                    redA[:],
                    tmpA[:].rearrange("p (g j) -> p j g", j=5),
                    _mb.AxisListType.X,
                    _mb.AluOpType.add,
                )
                o1 = work.tile([BL, 5], f32)
                nc.vector.tensor_add(o1[:], pB[:], redA[:])
                o2 = work.tile([BL, 5], f32)
                nc.vector.tensor_add(o2[:], o1[:], bro_sb[:])
                nc.sync.dma_start(out.ap(), o2[:])

    nc.compile()
    return nc


def _softmax(x):
    e = np.exp(x - x.max())
    return e / e.sum()


def _prepare_inputs(hidden_states, attention_mask, section_id, layer_weights,
                    W_down, b_down, W_up, b_up, W_reg, b_reg, W_ord, b_ord):
    hidden_states = np.asarray(hidden_states, dtype=np.float32)
    mask = np.asarray(attention_mask)
    sec = np.asarray(section_id).astype(np.int64)
    lw = np.asarray(layer_weights, dtype=np.float32)
    W_down = np.asarray(W_down, dtype=np.float32)
    b_down = np.asarray(b_down, dtype=np.float32)
    W_up = np.asarray(W_up, dtype=np.float32)
    b_up = np.asarray(b_up, dtype=np.float32)
    W_reg = np.asarray(W_reg, dtype=np.float32)
    b_reg = np.asarray(b_reg, dtype=np.float32)
    W_ord = np.asarray(W_ord, dtype=np.float32)
    b_ord = np.asarray(b_ord, dtype=np.float32)

    w = _softmax(lw)                                     # [L]
    mf = mask.astype(np.float32)                         # [B, S]
    msum = np.maximum(mf.sum(axis=1), 1e-6)              # [B]
    cmask = mf / msum[:, None]                           # [B, S]
    base = cmask.reshape(B, 2, 128).transpose(2, 1, 0)   # [p, c, b]
    coeff_all = w[None, :, None, None] * base[:, None, :, :]   # [p, l, c, b]

    wro_np = np.concatenate([W_reg, W_ord], axis=1)      # [H, 5]
    wro_dev = np.ascontiguousarray(wro_np.reshape(128, NH * 5))
    wd_all = W_down[sec]                                 # [B, H, K]
    # [p, r, b, k] with h = 8p + r
    wd_dev_all = wd_all.reshape(B, 128, NH, K).transpose(1, 2, 0, 3)
    wu_ro = np.einsum("ekh,hj->ekj", W_up, wro_np)       # [NSEC, K, 5]
    wuro_all = wu_ro[sec].transpose(1, 0, 2)             # [K, B, 5]
    bd_all = b_down[sec]                                 # [B, K]
    bro_all = b_up[sec] @ wro_np + np.concatenate([b_reg, b_ord])[None]  # [B, 5]

    dmask64_np = np.zeros((BL, BL * K), np.float32)
    dmask5_np = np.zeros((BL, BL * 5), np.float32)
    for i in range(BL):
        dmask64_np[i, i * K:(i + 1) * K] = 1.0
        dmask5_np[i, i * 5:(i + 1) * 5] = 1.0
    ident = np.eye(BL, dtype=np.float32)

    in_maps = []
    idx = np.arange(BL)
    for core in range(NCORES):
        sl = slice(core * BL, (core + 1) * BL)
        cc = np.zeros((128, L, 2, BL, BL), np.float32)
        cc[:, :, :, idx, idx] = coeff_all[:, :, :, sl]
        in_maps.append({
            "hs": np.ascontiguousarray(hidden_states[:, sl]),
            "coeff": np.ascontiguousarray(cc.reshape(128, L * 2 * BL * BL)),
            "wd": np.ascontiguousarray(
                wd_dev_all[:, :, sl].reshape(128, NH * BL * K)),
            "wro": wro_dev,
            "wuro": np.ascontiguousarray(wuro_all[:, sl].reshape(K, BL * 5)),
            "dmask64": dmask64_np,
            "dmask5": dmask5_np,
            "bd": np.ascontiguousarray(bd_all[sl]),
            "bro": np.ascontiguousarray(bro_all[sl]),
            "ident": ident,
        })
    return in_maps


def get_module(act_fn="Gelu"):
    key = "nc_" + act_fn
    if key not in _CACHE:
        _CACHE[key] = _build_module(act_fn)
    return _CACHE[key]


def kernel(hidden_states, attention_mask, section_id, layer_weights,
           W_down, b_down, W_up, b_up, W_reg, b_reg, W_ord, b_ord):
    global LAST_RESULT
    from concourse.bass_utils import run_bass_kernel_spmd

    in_maps = _prepare_inputs(
        hidden_states, attention_mask, section_id, layer_weights,
        W_down, b_down, W_up, b_up, W_reg, b_reg, W_ord, b_ord)
    nc = get_module()
    res = run_bass_kernel_spmd(nc, in_maps, list(range(NCORES)))
    LAST_RESULT = res
    out = np.concatenate([res.results[c]["out"] for c in range(NCORES)], axis=0)
    reg = np.ascontiguousarray(out[:, 0])
    ord_logits = np.ascontiguousarray(out[:, 1:5])
    return reg, ord_logits


# revision 9
# speedup vs baseline: 1.0678x; 1.0678x over previous
"""Trainium2 Bass kernel for CrossEncoderMTL (weighted layer pooling + masked mean
+ section-routed adapter + reg/ord heads), data-parallel over batch across 8 cores.

Self-contained: hardcodes shapes; host-side numpy does sharding + index gathers;
the device kernel does all the heavy compute (streaming the 1 GiB hidden_states).
"""

import numpy as np

L, B, S, H, K = 4, 256, 256, 1024, 64
NCORES = 8
BL = B // NCORES          # 32 samples per core
NH = H // 128             # 8 h-chunks (h = 8*p + r mapping)
BPAIR = 2                 # samples per hidden_states DMA (2 MiB chunks)

_CACHE = {}
LAST_RESULT = None        # BassKernelResults of the most recent run (for profiling)


def _build_module(act_fn="Gelu"):
    from contextlib import ExitStack
    from concourse import bacc, mybir, tile

    f32 = mybir.dt.float32
    nc = bacc.Bacc(
        "TRN2", target_bir_lowering=False, debug=False, num_devices=NCORES
    )

    # ---- DRAM I/O (per-core shapes; device layouts precomputed on host) ----
    hs = nc.dram_tensor("hs", [L, BL, S, H], f32, kind="ExternalInput")
    # coeff[p, ((l*2+c)*BL + b)*BL + m] = softmax(w)[l]*mask[b,c*128+p]/msum[b]
    #                                     if m==b else 0
    coeff = nc.dram_tensor("coeff", [128, L * 2 * BL * BL], f32, kind="ExternalInput")
    # wd[p, (r*BL + b)*K + k] = W_down[sec[b], 8p+r, k]
    wd = nc.dram_tensor("wd", [128, NH * BL * K], f32, kind="ExternalInput")
    # wro[p, r*5+j] = [W_reg | W_ord][8p+r, j]
    wro = nc.dram_tensor("wro", [128, NH * 5], f32, kind="ExternalInput")
    # wuro[k, b*5+j] = (W_up[sec[b]] @ [W_reg|W_ord])[k, j]
    wuro = nc.dram_tensor("wuro", [K, BL * 5], f32, kind="ExternalInput")
    # dmask64[m, b*64+k] = 1.0 iff m == b   (diagonal extraction, down-proj)
    dmask64 = nc.dram_tensor("dmask64", [BL, BL * K], f32, kind="ExternalInput")
    # dmask5[m, b*5+j] = 1.0 iff m == b     (diagonal extraction, head A)
    dmask5 = nc.dram_tensor("dmask5", [BL, BL * 5], f32, kind="ExternalInput")
    # bd[b, k] = b_down[sec[b], k]
    bd = nc.dram_tensor("bd", [BL, K], f32, kind="ExternalInput")
    # bro[b, j] = (b_up[sec[b]] @ [W_reg|W_ord])[j] + [b_reg | b_ord][j]
    bro = nc.dram_tensor("bro", [BL, 5], f32, kind="ExternalInput")
    ident = nc.dram_tensor("ident", [BL, BL], f32, kind="ExternalInput")
    out = nc.dram_tensor("out", [BL, 5], f32, kind="ExternalOutput")

    with tile.TileContext(nc) as tc:
        with ExitStack() as ctx:
            consts = ctx.enter_context(tc.tile_pool(name="consts", bufs=1))
            hs_pool = ctx.enter_context(tc.tile_pool(name="hsp", bufs=4))
            work = ctx.enter_context(tc.tile_pool(name="work", bufs=1))

            # Constants go on the scalar-engine HWDGE ring so they never
            # head-of-line-block the hidden_states stream on the sync ring.
            def cload(dram, shape):
                t = consts.tile(shape, f32, tag=dram.name)
                nc.scalar.dma_start(t[:], dram.ap())
                return t

            id_sb = cload(ident, [BL, BL])
            wro_sb = cload(wro, [128, NH * 5])
            # Big constants are staggered into the pooling loop below so they
            # don't compete with the hidden_states stream at startup.
            coeff_sb = consts.tile([128, L * 2 * BL * BL], f32, tag="coeff")
            wd_sb = consts.tile([128, NH * BL * K], f32, tag="wd")
            wuro_sb = consts.tile([K, BL * 5], f32, tag="wuro")
            bd_sb = consts.tile([BL, K], f32, tag="bd")
            bro_sb = consts.tile([BL, 5], f32, tag="bro")
            dmask64_sb = consts.tile([BL, BL * K], f32, tag="dmask64")
            dmask5_sb = consts.tile([BL, BL * 5], f32, tag="dmask5")

            # All remaining constants go on the SYNC ring, interleaved with the
            # hs chunks: the sync HWDGE ring is a single FIFO queue, so this
            # staggers them temporally instead of competing at startup.
            def load_coeff_blocks(ll):
                for c in range(2):
                    j = ll * 2 + c
                    blk = slice(j * BL * BL, (j + 1) * BL * BL)
                    nc.sync.dma_start(coeff_sb[:, blk], coeff.ap()[:, blk])

            def load_wd_chunk(r):
                blk = slice(r * BL * K, (r + 1) * BL * K)
                nc.sync.dma_start(wd_sb[:, blk], wd.ap()[:, blk])

            load_coeff_blocks(0)

            # ---- PE warm-up: ~4us of tiny matmuls so HAM reaches K=8/8
            # before the real stream starts (runs during startup DMAs) ----
            with tc.tile_pool(name="pwarm", bufs=1, space="PSUM") as pwarm:
                wps = pwarm.tile([BL, BL], f32)
                for _ in range(48):
                    nc.tensor.matmul(wps[:], id_sb[:], id_sb[:],
                                     start=True, stop=True)

            # ---- pooling: feats[b, h] = sum_{l,s} w_l * c[b,s] * hs[l,b,s,h] ----
            # Zero-embedded coeff columns route sample b's contribution to psum
            # row b, so feats accumulates directly as [BL, H] with b on
            # partitions.
            feats_sb = work.tile([BL, H], f32)
            with tc.tile_pool(name="pf", bufs=1, space="PSUM") as pf_pool:
                pf = pf_pool.tile([BL, H], f32)
                hs_ap = hs.ap()
                idx = 0
                n_idx = L * BL * 2
                for l in range(L):
                    for bp in range(BL // BPAIR):
                        # Interleave late-needed constants into the sync-ring
                        # FIFO, one layer (~110us) ahead of their use.
                        if bp == 2 and l + 1 < L:
                            load_coeff_blocks(l + 1)
                        if l >= 2 and bp in (4, 8, 12, 15):
                            load_wd_chunk((l - 2) * 4 + (4, 8, 12, 15).index(bp))
                        if l == 2 and bp == 6:
                            nc.sync.dma_start(wuro_sb[:], wuro.ap())
                            nc.sync.dma_start(bd_sb[:], bd.ap())
                            nc.sync.dma_start(bro_sb[:], bro.ap())
                        if l == 3 and bp == 6:
                            nc.sync.dma_start(dmask64_sb[:], dmask64.ap())
                            nc.sync.dma_start(dmask5_sb[:], dmask5.ap())
                        t = hs_pool.tile([128, BPAIR * 2 * 1024], f32, tag="hst")
                        src = hs_ap[l, bp * BPAIR:(bp + 1) * BPAIR].rearrange(
                            "b (c p) h -> p b c h", p=128
                        )
                        dst = t[:].rearrange("p (b c h) -> p b c h", b=BPAIR, c=2)
                        nc.sync.dma_start(dst, src)
                        for bb in range(BPAIR):
                            b = bp * BPAIR + bb
                            for c in range(2):
                                j = l * 2 + c
                                lhs = coeff_sb[
                                    :, (j * BL + b) * BL:(j * BL + b + 1) * BL]
                                for hh in range(2):
                                    base = bb * 2048 + c * 1024 + hh * 512
                                    nc.tensor.matmul(
                                        pf[:, hh * 512:(hh + 1) * 512],
                                        lhs,
                                        t[:, base:base + 512],
                                        start=(idx == 0),
                                        stop=(idx == n_idx - 1),
                                    )
                                idx += 1
                nc.vector.tensor_copy(feats_sb[:], pf[:])
            # pf's 2 PSUM banks are free again here.

            from concourse import mybir as _mb
            with tc.tile_pool(name="pt", bufs=2, space="PSUM") as pt_pool, \
                 tc.tile_pool(name="pd", bufs=1, space="PSUM") as pd_pool, \
                 tc.tile_pool(name="ph", bufs=1, space="PSUM") as ph_pool:
                # ---- transpose feats to h-on-partitions ----
                # featsT[:, r*BL + b][p] = feats[b, 8p + r]
                featsT = work.tile([128, NH * BL], f32)
                fview = feats_sb[:].rearrange("p (q r) -> p r q", r=NH)
                for r in range(NH):
                    pt = pt_pool.tile([128, BL], f32, tag="pt")
                    nc.tensor.transpose(pt[:], fview[:, r], id_sb[:])
                    nc.vector.tensor_copy(featsT[:, r * BL:(r + 1) * BL], pt[:])

                # ---- batched down-projection ----
                # bigD[m, b*K+k] = sum_h feats[m,h] * W_down[sec[b],h,k];
                # the diagonal m==b is the wanted z[b, k].
                bigD = pd_pool.tile([BL, BL * K], f32, tag="bigD")
                nmm = BL * K // 512                      # 4 matmuls of N=512
                for r in range(NH):
                    lhsT = featsT[:, r * BL:(r + 1) * BL]
                    for n in range(nmm):
                        nc.tensor.matmul(
                            bigD[:, n * 512:(n + 1) * 512],
                            lhsT,
                            wd_sb[:, r * BL * K + n * 512:
                                  r * BL * K + (n + 1) * 512],
                            start=(r == 0),
                            stop=(r == NH - 1),
                        )
                zmask = work.tile([BL, BL * K], f32)
                nc.vector.tensor_mul(zmask[:], bigD[:], dmask64_sb[:])
                z_bt = work.tile([BL, K], f32)
                nc.vector.tensor_reduce(
                    z_bt[:],
                    zmask[:].rearrange("p (g j) -> p j g", j=K),
                    _mb.AxisListType.X,
                    _mb.AluOpType.add,
                )
                zb = work.tile([BL, K], f32)
                nc.vector.tensor_add(zb[:], z_bt[:], bd_sb[:])
                h1_bt = work.tile([BL, K], f32)
                nc.scalar.activation(
                    h1_bt[:], zb[:], getattr(_mb.ActivationFunctionType, act_fn))
                # h1 with k on partitions for the head-A matmul
                pth = pt_pool.tile([K, BL], f32, tag="pt")
                nc.tensor.transpose(pth[:], h1_bt[:], id_sb[:])
                h1_sb = work.tile([K, BL], f32)
                nc.vector.tensor_copy(h1_sb[:], pth[:])

                # ---- heads ----
                # B-part: feats @ [W_reg|W_ord] -> pB[b, j]
                pB = ph_pool.tile([BL, 5], f32, tag="pB")
                for r in range(NH):
                    nc.tensor.matmul(
                        pB[:],
                        featsT[:, r * BL:(r + 1) * BL],
                        wro_sb[:, r * 5:(r + 1) * 5],
                        start=(r == 0),
                        stop=(r == NH - 1),
                    )
                # A-part: h1[b] @ (W_up[sec[b]] @ W_ro) for all (m, b) pairs,
                # then diagonal-extract.
                pA = ph_pool.tile([BL, BL * 5], f32, tag="pA")
                nc.tensor.matmul(pA[:], h1_sb[:], wuro_sb[:], start=True, stop=True)
                tmpA = work.tile([BL, BL * 5], f32)
                nc.vector.tensor_mul(tmpA[:], pA[:], dmask5_sb[:])
                redA = work.tile([BL, 5], f32)
                nc.vector.tensor_reduce(
                    redA[:],
                    tmpA[:].rearrange("p (g j) -> p j g", j=5),
                    _mb.AxisListType.X,
                    _mb.AluOpType.add,
                )
                o1 = work.tile([BL, 5], f32)
                nc.vector.tensor_add(o1[:], pB[:], redA[:])
                o2 = work.tile([BL, 5], f32)
                nc.vector.tensor_add(o2[:], o1[:], bro_sb[:])
                nc.sync.dma_start(out.ap(), o2[:])

    nc.compile()
    return nc


def _softmax(x):
    e = np.exp(x - x.max())
    return e / e.sum()


def _prepare_inputs(hidden_states, attention_mask, section_id, layer_weights,
                    W_down, b_down, W_up, b_up, W_reg, b_reg, W_ord, b_ord):
    hidden_states = np.asarray(hidden_states, dtype=np.float32)
    mask = np.asarray(attention_mask)
    sec = np.asarray(section_id).astype(np.int64)
    lw = np.asarray(layer_weights, dtype=np.float32)
    W_down = np.asarray(W_down, dtype=np.float32)
    b_down = np.asarray(b_down, dtype=np.float32)
    W_up = np.asarray(W_up, dtype=np.float32)
    b_up = np.asarray(b_up, dtype=np.float32)
    W_reg = np.asarray(W_reg, dtype=np.float32)
    b_reg = np.asarray(b_reg, dtype=np.float32)
    W_ord = np.asarray(W_ord, dtype=np.float32)
    b_ord = np.asarray(b_ord, dtype=np.float32)

    w = _softmax(lw)                                     # [L]
    mf = mask.astype(np.float32)                         # [B, S]
    msum = np.maximum(mf.sum(axis=1), 1e-6)              # [B]
    cmask = mf / msum[:, None]                           # [B, S]
    base = cmask.reshape(B, 2, 128).transpose(2, 1, 0)   # [p, c, b]
    coeff_all = w[None, :, None, None] * base[:, None, :, :]   # [p, l, c, b]

    wro_np = np.concatenate([W_reg, W_ord], axis=1)      # [H, 5]
    wro_dev = np.ascontiguousarray(wro_np.reshape(128, NH * 5))
    wd_all = W_down[sec]                                 # [B, H, K]
    # [p, r, b, k] with h = 8p + r
    wd_dev_all = wd_all.reshape(B, 128, NH, K).transpose(1, 2, 0, 3)
    wu_ro = np.einsum("ekh,hj->ekj", W_up, wro_np)       # [NSEC, K, 5]
    wuro_all = wu_ro[sec].transpose(1, 0, 2)             # [K, B, 5]
    bd_all = b_down[sec]                                 # [B, K]
    bro_all = b_up[sec] @ wro_np + np.concatenate([b_reg, b_ord])[None]  # [B, 5]

    dmask64_np = np.zeros((BL, BL * K), np.float32)
    dmask5_np = np.zeros((BL, BL * 5), np.float32)
    for i in range(BL):
        dmask64_np[i, i * K:(i + 1) * K] = 1.0
        dmask5_np[i, i * 5:(i + 1) * 5] = 1.0
    ident = np.eye(BL, dtype=np.float32)

    in_maps = []
    idx = np.arange(BL)
    for core in range(NCORES):
        sl = slice(core * BL, (core + 1) * BL)
        cc = np.zeros((128, L, 2, BL, BL), np.float32)
        cc[:, :, :, idx, idx] = coeff_all[:, :, :, sl]
        in_maps.append({
            "hs": np.ascontiguousarray(hidden_states[:, sl]),
            "coeff": np.ascontiguousarray(cc.reshape(128, L * 2 * BL * BL)),
            "wd": np.ascontiguousarray(
                wd_dev_all[:, :, sl].reshape(128, NH * BL * K)),
            "wro": wro_dev,
            "wuro": np.ascontiguousarray(wuro_all[:, sl].reshape(K, BL * 5)),
            "dmask64": dmask64_np,
            "dmask5": dmask5_np,
            "bd": np.ascontiguousarray(bd_all[sl]),
            "bro": np.ascontiguousarray(bro_all[sl]),
            "ident": ident,
        })
    return in_maps


def get_module(act_fn="Gelu"):
    key = "nc_" + act_fn
    if key not in _CACHE:
        _CACHE[key] = _build_module(act_fn)
    return _CACHE[key]


def kernel(hidden_states, attention_mask, section_id, layer_weights,
           W_down, b_down, W_up, b_up, W_reg, b_reg, W_ord, b_ord):
    global LAST_RESULT
    from concourse.bass_utils import run_bass_kernel_spmd

    in_maps = _prepare_inputs(
        hidden_states, attention_mask, section_id, layer_weights,
        W_down, b_down, W_up, b_up, W_reg, b_reg, W_ord, b_ord)
    nc = get_module()
    res = run_bass_kernel_spmd(nc, in_maps, list(range(NCORES)))
    LAST_RESULT = res
    out = np.concatenate([res.results[c]["out"] for c in range(NCORES)], axis=0)
    reg = np.ascontiguousarray(out[:, 0])
    ord_logits = np.ascontiguousarray(out[:, 1:5])
    return reg, ord_logits


# revision 14
# speedup vs baseline: 1.1379x; 1.0657x over previous
"""Trainium2 Bass kernel for CrossEncoderMTL (weighted layer pooling + masked mean
+ section-routed adapter + reg/ord heads), data-parallel over batch across 8 cores.

Self-contained: hardcodes shapes; host-side numpy does sharding + index gathers;
the device kernel does all the heavy compute (streaming the 1 GiB hidden_states).
"""

import numpy as np

L, B, S, H, K = 4, 256, 256, 1024, 64
NCORES = 8
BL = B // NCORES          # 32 samples per core
NH = H // 128             # 8 h-chunks (h = 8*p + r mapping)
BPAIR = 2                 # samples per hidden_states DMA (2 MiB chunks)

_CACHE = {}
LAST_RESULT = None        # BassKernelResults of the most recent run (for profiling)


def _build_module(act_fn="Gelu", mm_dtype="float32r"):
    from contextlib import ExitStack
    from concourse import bacc, mybir, tile

    f32 = mybir.dt.float32
    # Stream/weight tensors of the two big matmul stages use fp32r: same bytes
    # as fp32, but the PE runs single-pass (4x fp32 throughput) at slightly
    # reduced multiply precision. PSUM accumulation stays full fp32.
    fmm = getattr(mybir.dt, mm_dtype)
    nc = bacc.Bacc(
        "TRN2", target_bir_lowering=False, debug=False, num_devices=NCORES
    )

    # ---- DRAM I/O (per-core shapes; device layouts precomputed on host) ----
    hs = nc.dram_tensor("hs", [L, BL, S, H], fmm, kind="ExternalInput")
    # coeff[p, ((l*2+c)*BL + b)*BL + m] = softmax(w)[l]*mask[b,c*128+p]/msum[b]
    #                                     if m==b else 0
    coeff = nc.dram_tensor("coeff", [128, L * 2 * BL * BL], fmm, kind="ExternalInput")
    # wd[p, (r*BL + b)*K + k] = W_down[sec[b], 8p+r, k]
    wd = nc.dram_tensor("wd", [128, NH * BL * K], fmm, kind="ExternalInput")
    # wro[p, r*5+j] = [W_reg | W_ord][8p+r, j]
    wro = nc.dram_tensor("wro", [128, NH * 5], f32, kind="ExternalInput")
    # wuro[k, b*5+j] = (W_up[sec[b]] @ [W_reg|W_ord])[k, j]
    wuro = nc.dram_tensor("wuro", [K, BL * 5], f32, kind="ExternalInput")
    # dmask64[m, b*64+k] = 1.0 iff m == b   (diagonal extraction, down-proj)
    dmask64 = nc.dram_tensor("dmask64", [BL, BL * K], f32, kind="ExternalInput")
    # dmask5[m, b*5+j] = 1.0 iff m == b     (diagonal extraction, head A)
    dmask5 = nc.dram_tensor("dmask5", [BL, BL * 5], f32, kind="ExternalInput")
    # bd[b, k] = b_down[sec[b], k]
    bd = nc.dram_tensor("bd", [BL, K], f32, kind="ExternalInput")
    # bro[b, j] = (b_up[sec[b]] @ [W_reg|W_ord])[j] + [b_reg | b_ord][j]
    bro = nc.dram_tensor("bro", [BL, 5], f32, kind="ExternalInput")
    ident = nc.dram_tensor("ident", [BL, BL], f32, kind="ExternalInput")
    out = nc.dram_tensor("out", [BL, 5], f32, kind="ExternalOutput")

    with tile.TileContext(nc) as tc:
        with ExitStack() as ctx:
            consts = ctx.enter_context(tc.tile_pool(name="consts", bufs=1))
            hs_pool = ctx.enter_context(tc.tile_pool(name="hsp", bufs=4))
            work = ctx.enter_context(tc.tile_pool(name="work", bufs=1))

            # Constants go on the scalar-engine HWDGE ring so they never
            # head-of-line-block the hidden_states stream on the sync ring.
            def cload(dram, shape, dt=f32):
                t = consts.tile(shape, dt, tag=dram.name)
                nc.scalar.dma_start(t[:], dram.ap())
                return t

            id_sb = cload(ident, [BL, BL])
            wro_sb = cload(wro, [128, NH * 5])
            # Big constants are staggered into the pooling loop below so they
            # don't compete with the hidden_states stream at startup.
            coeff_sb = consts.tile([128, L * 2 * BL * BL], fmm, tag="coeff")
            wd_sb = consts.tile([128, NH * BL * K], fmm, tag="wd")
            wuro_sb = consts.tile([K, BL * 5], f32, tag="wuro")
            bd_sb = consts.tile([BL, K], f32, tag="bd")
            bro_sb = consts.tile([BL, 5], f32, tag="bro")
            dmask64_sb = consts.tile([BL, BL * K], f32, tag="dmask64")
            dmask5_sb = consts.tile([BL, BL * 5], f32, tag="dmask5")

            # All remaining constants go on the SYNC ring, interleaved with the
            # hs chunks: the sync HWDGE ring is a single FIFO queue, so this
            # staggers them temporally instead of competing at startup.
            def load_coeff_blocks(ll):
                for c in range(2):
                    j = ll * 2 + c
                    blk = slice(j * BL * BL, (j + 1) * BL * BL)
                    nc.sync.dma_start(coeff_sb[:, blk], coeff.ap()[:, blk])

            def load_wd_chunk(r):
                blk = slice(r * BL * K, (r + 1) * BL * K)
                nc.sync.dma_start(wd_sb[:, blk], wd.ap()[:, blk])

            load_coeff_blocks(0)

            # ---- PE warm-up: ~4us of tiny matmuls so HAM reaches K=8/8
            # before the real stream starts (runs during startup DMAs) ----
            with tc.tile_pool(name="pwarm", bufs=1, space="PSUM") as pwarm:
                wps = pwarm.tile([BL, BL], f32)
                for _ in range(48):
                    nc.tensor.matmul(wps[:], id_sb[:], id_sb[:],
                                     start=True, stop=True)

            # ---- pooling: feats[b, h] = sum_{l,s} w_l * c[b,s] * hs[l,b,s,h] ----
            # Zero-embedded coeff columns route sample b's contribution to psum
            # row b, so feats accumulates directly as [BL, H] with b on
            # partitions.
            feats_sb = work.tile([BL, H], f32)
            with tc.tile_pool(name="pf", bufs=1, space="PSUM") as pf_pool:
                pf = pf_pool.tile([BL, H], f32)
                hs_ap = hs.ap()
                idx = 0
                n_idx = L * BL * 2
                for l in range(L):
                    for bp in range(BL // BPAIR):
                        # Interleave late-needed constants into the sync-ring
                        # FIFO, one layer (~110us) ahead of their use.
                        if bp == 2 and l + 1 < L:
                            load_coeff_blocks(l + 1)
                        if l >= 2 and bp in (4, 8, 12, 15):
                            load_wd_chunk((l - 2) * 4 + (4, 8, 12, 15).index(bp))
                        if l == 2 and bp == 6:
                            nc.sync.dma_start(wuro_sb[:], wuro.ap())
                            nc.sync.dma_start(bd_sb[:], bd.ap())
                            nc.sync.dma_start(bro_sb[:], bro.ap())
                        if l == 3 and bp == 6:
                            nc.sync.dma_start(dmask64_sb[:], dmask64.ap())
                            nc.sync.dma_start(dmask5_sb[:], dmask5.ap())
                        t = hs_pool.tile([128, BPAIR * 2 * 1024], fmm, tag="hst")
                        src = hs_ap[l, bp * BPAIR:(bp + 1) * BPAIR].rearrange(
                            "b (c p) h -> p b c h", p=128
                        )
                        dst = t[:].rearrange("p (b c h) -> p b c h", b=BPAIR, c=2)
                        nc.sync.dma_start(dst, src)
                        for bb in range(BPAIR):
                            b = bp * BPAIR + bb
                            for c in range(2):
                                j = l * 2 + c
                                lhs = coeff_sb[
                                    :, (j * BL + b) * BL:(j * BL + b + 1) * BL]
                                for hh in range(2):
                                    base = bb * 2048 + c * 1024 + hh * 512
                                    nc.tensor.matmul(
                                        pf[:, hh * 512:(hh + 1) * 512],
                                        lhs,
                                        t[:, base:base + 512],
                                        start=(idx == 0),
                                        stop=(idx == n_idx - 1),
                                    )
                                idx += 1
                nc.vector.tensor_copy(feats_sb[:], pf[:])
            # pf's 2 PSUM banks are free again here.

            from concourse import mybir as _mb
            with tc.tile_pool(name="pt", bufs=2, space="PSUM") as pt_pool, \
                 tc.tile_pool(name="pd", bufs=1, space="PSUM") as pd_pool, \
                 tc.tile_pool(name="ph", bufs=1, space="PSUM") as ph_pool:
                # ---- transpose feats to h-on-partitions ----
                # featsT[:, r*BL + b][p] = feats[b, 8p + r]
                featsT = work.tile([128, NH * BL], fmm)
                fview = feats_sb[:].rearrange("p (q r) -> p r q", r=NH)
                for r in range(NH):
                    pt = pt_pool.tile([128, BL], f32, tag="pt")
                    nc.tensor.transpose(pt[:], fview[:, r], id_sb[:])
                    nc.vector.tensor_copy(featsT[:, r * BL:(r + 1) * BL], pt[:])

                # ---- batched down-projection ----
                # bigD[m, b*K+k] = sum_h feats[m,h] * W_down[sec[b],h,k];
                # the diagonal m==b is the wanted z[b, k].
                bigD = pd_pool.tile([BL, BL * K], f32, tag="bigD")
                nmm = BL * K // 512                      # 4 matmuls of N=512
                for r in range(NH):
                    lhsT = featsT[:, r * BL:(r + 1) * BL]
                    for n in range(nmm):
                        nc.tensor.matmul(
                            bigD[:, n * 512:(n + 1) * 512],
                            lhsT,
                            wd_sb[:, r * BL * K + n * 512:
                                  r * BL * K + (n + 1) * 512],
                            start=(r == 0),
                            stop=(r == NH - 1),
                        )
                zmask = work.tile([BL, BL * K], f32)
                nc.vector.tensor_mul(zmask[:], bigD[:], dmask64_sb[:])
                z_bt = work.tile([BL, K], f32)
                nc.vector.tensor_reduce(
                    z_bt[:],
                    zmask[:].rearrange("p (g j) -> p j g", j=K),
                    _mb.AxisListType.X,
                    _mb.AluOpType.add,
                )
                zb = work.tile([BL, K], f32)
                nc.vector.tensor_add(zb[:], z_bt[:], bd_sb[:])
                h1_bt = work.tile([BL, K], f32)
                nc.scalar.activation(
                    h1_bt[:], zb[:], getattr(_mb.ActivationFunctionType, act_fn))
                # h1 with k on partitions for the head-A matmul
                pth = pt_pool.tile([K, BL], f32, tag="pt")
                nc.tensor.transpose(pth[:], h1_bt[:], id_sb[:])
                h1_sb = work.tile([K, BL], f32)
                nc.vector.tensor_copy(h1_sb[:], pth[:])

                # ---- heads ----
                # B-part: feats @ [W_reg|W_ord] -> pB[b, j]
                pB = ph_pool.tile([BL, 5], f32, tag="pB")
                for r in range(NH):
                    # head-B stays fp32 (fp32r rejects tiny moving dims)
                    nc.tensor.matmul(
                        pB[:],
                        featsT[:, r * BL:(r + 1) * BL].bitcast(f32),
                        wro_sb[:, r * 5:(r + 1) * 5],
                        start=(r == 0),
                        stop=(r == NH - 1),
                    )
                # A-part: h1[b] @ (W_up[sec[b]] @ W_ro) for all (m, b) pairs,
                # then diagonal-extract.
                pA = ph_pool.tile([BL, BL * 5], f32, tag="pA")
                nc.tensor.matmul(pA[:], h1_sb[:], wuro_sb[:], start=True, stop=True)
                tmpA = work.tile([BL, BL * 5], f32)
                nc.vector.tensor_mul(tmpA[:], pA[:], dmask5_sb[:])
                redA = work.tile([BL, 5], f32)
                nc.vector.tensor_reduce(
                    redA[:],
                    tmpA[:].rearrange("p (g j) -> p j g", j=5),
                    _mb.AxisListType.X,
                    _mb.AluOpType.add,
                )
                o1 = work.tile([BL, 5], f32)
                nc.vector.tensor_add(o1[:], pB[:], redA[:])
                o2 = work.tile([BL, 5], f32)
                nc.vector.tensor_add(o2[:], o1[:], bro_sb[:])
                nc.sync.dma_start(out.ap(), o2[:])

    nc.compile()
    return nc


def _softmax(x):
    e = np.exp(x - x.max())
    return e / e.sum()


def _prepare_inputs(hidden_states, attention_mask, section_id, layer_weights,
                    W_down, b_down, W_up, b_up, W_reg, b_reg, W_ord, b_ord):
    hidden_states = np.asarray(hidden_states, dtype=np.float32)
    mask = np.asarray(attention_mask)
    sec = np.asarray(section_id).astype(np.int64)
    lw = np.asarray(layer_weights, dtype=np.float32)
    W_down = np.asarray(W_down, dtype=np.float32)
    b_down = np.asarray(b_down, dtype=np.float32)
    W_up = np.asarray(W_up, dtype=np.float32)
    b_up = np.asarray(b_up, dtype=np.float32)
    W_reg = np.asarray(W_reg, dtype=np.float32)
    b_reg = np.asarray(b_reg, dtype=np.float32)
    W_ord = np.asarray(W_ord, dtype=np.float32)
    b_ord = np.asarray(b_ord, dtype=np.float32)

    w = _softmax(lw)                                     # [L]
    mf = mask.astype(np.float32)                         # [B, S]
    msum = np.maximum(mf.sum(axis=1), 1e-6)              # [B]
    cmask = mf / msum[:, None]                           # [B, S]
    base = cmask.reshape(B, 2, 128).transpose(2, 1, 0)   # [p, c, b]
    coeff_all = w[None, :, None, None] * base[:, None, :, :]   # [p, l, c, b]

    wro_np = np.concatenate([W_reg, W_ord], axis=1)      # [H, 5]
    wro_dev = np.ascontiguousarray(wro_np.reshape(128, NH * 5))
    wd_all = W_down[sec]                                 # [B, H, K]
    # [p, r, b, k] with h = 8p + r
    wd_dev_all = wd_all.reshape(B, 128, NH, K).transpose(1, 2, 0, 3)
    wu_ro = np.einsum("ekh,hj->ekj", W_up, wro_np)       # [NSEC, K, 5]
    wuro_all = wu_ro[sec].transpose(1, 0, 2)             # [K, B, 5]
    bd_all = b_down[sec]                                 # [B, K]
    bro_all = b_up[sec] @ wro_np + np.concatenate([b_reg, b_ord])[None]  # [B, 5]

    dmask64_np = np.zeros((BL, BL * K), np.float32)
    dmask5_np = np.zeros((BL, BL * 5), np.float32)
    for i in range(BL):
        dmask64_np[i, i * K:(i + 1) * K] = 1.0
        dmask5_np[i, i * 5:(i + 1) * 5] = 1.0
    ident = np.eye(BL, dtype=np.float32)

    in_maps = []
    idx = np.arange(BL)
    for core in range(NCORES):
        sl = slice(core * BL, (core + 1) * BL)
        cc = np.zeros((128, L, 2, BL, BL), np.float32)
        cc[:, :, :, idx, idx] = coeff_all[:, :, :, sl]
        in_maps.append({
            "hs": np.ascontiguousarray(hidden_states[:, sl]),
            "coeff": np.ascontiguousarray(cc.reshape(128, L * 2 * BL * BL)),
            "wd": np.ascontiguousarray(
                wd_dev_all[:, :, sl].reshape(128, NH * BL * K)),
            "wro": wro_dev,
            "wuro": np.ascontiguousarray(wuro_all[:, sl].reshape(K, BL * 5)),
            "dmask64": dmask64_np,
            "dmask5": dmask5_np,
            "bd": np.ascontiguousarray(bd_all[sl]),
            "bro": np.ascontiguousarray(bro_all[sl]),
            "ident": ident,
        })
    return in_maps


def get_module(act_fn="Gelu"):
    key = "nc_" + act_fn
    if key not in _CACHE:
        _CACHE[key] = _build_module(act_fn)
    return _CACHE[key]


def kernel(hidden_states, attention_mask, section_id, layer_weights,
           W_down, b_down, W_up, b_up, W_reg, b_reg, W_ord, b_ord):
    global LAST_RESULT
    from concourse.bass_utils import run_bass_kernel_spmd

    in_maps = _prepare_inputs(
        hidden_states, attention_mask, section_id, layer_weights,
        W_down, b_down, W_up, b_up, W_reg, b_reg, W_ord, b_ord)
    nc = get_module()
    res = run_bass_kernel_spmd(nc, in_maps, list(range(NCORES)))
    LAST_RESULT = res
    out = np.concatenate([res.results[c]["out"] for c in range(NCORES)], axis=0)
    reg = np.ascontiguousarray(out[:, 0])
    ord_logits = np.ascontiguousarray(out[:, 1:5])
    return reg, ord_logits


# revision 15
# speedup vs baseline: 1.2352x; 1.0854x over previous
"""Trainium2 Bass kernel for CrossEncoderMTL (weighted layer pooling + masked mean
+ section-routed adapter + reg/ord heads), data-parallel over batch across 8 cores.

Self-contained: hardcodes shapes; host-side numpy does sharding + index gathers;
the device kernel does all the heavy compute (streaming the 1 GiB hidden_states).
"""

import numpy as np

L, B, S, H, K = 4, 256, 256, 1024, 64
NCORES = 8
BL = B // NCORES          # 32 samples per core
NH = H // 128             # 8 h-chunks (h = 8*p + r mapping)
BPAIR = 2                 # samples per hidden_states DMA (2 MiB chunks)

_CACHE = {}
LAST_RESULT = None        # BassKernelResults of the most recent run (for profiling)


def _build_module(act_fn="Gelu", mm_dtype="float32"):
    from contextlib import ExitStack
    from concourse import bacc, mybir, tile

    f32 = mybir.dt.float32
    # Stream/weight tensors of the two big matmul stages use fp32r: same bytes
    # as fp32, but the PE runs single-pass (4x fp32 throughput) at slightly
    # reduced multiply precision. PSUM accumulation stays full fp32.
    fmm = getattr(mybir.dt, mm_dtype)
    nc = bacc.Bacc(
        "TRN2", target_bir_lowering=False, debug=False, num_devices=NCORES
    )

    # ---- DRAM I/O (per-core shapes; device layouts precomputed on host) ----
    hs = nc.dram_tensor("hs", [L, BL, S, H], fmm, kind="ExternalInput")
    # coeff[p, ((l*2+c)*BL + b)*BL + m] = softmax(w)[l]*mask[b,c*128+p]/msum[b]
    #                                     if m==b else 0
    coeff = nc.dram_tensor("coeff", [128, L * 2 * BL * BL], fmm, kind="ExternalInput")
    # wd[p, (r*BL + b)*K + k] = W_down[sec[b], 8p+r, k]
    wd = nc.dram_tensor("wd", [128, NH * BL * K], fmm, kind="ExternalInput")
    # wro[p, r*5+j] = [W_reg | W_ord][8p+r, j]
    wro = nc.dram_tensor("wro", [128, NH * 5], f32, kind="ExternalInput")
    # wuro[k, b*5+j] = (W_up[sec[b]] @ [W_reg|W_ord])[k, j]
    wuro = nc.dram_tensor("wuro", [K, BL * 5], f32, kind="ExternalInput")
    # dmask64[m, b*64+k] = 1.0 iff m == b   (diagonal extraction, down-proj)
    dmask64 = nc.dram_tensor("dmask64", [BL, BL * K], f32, kind="ExternalInput")
    # dmask5[m, b*5+j] = 1.0 iff m == b     (diagonal extraction, head A)
    dmask5 = nc.dram_tensor("dmask5", [BL, BL * 5], f32, kind="ExternalInput")
    # bd[b, k] = b_down[sec[b], k]
    bd = nc.dram_tensor("bd", [BL, K], f32, kind="ExternalInput")
    # bro[b, j] = (b_up[sec[b]] @ [W_reg|W_ord])[j] + [b_reg | b_ord][j]
    bro = nc.dram_tensor("bro", [BL, 5], f32, kind="ExternalInput")
    ident = nc.dram_tensor("ident", [BL, BL], f32, kind="ExternalInput")
    out = nc.dram_tensor("out", [BL, 5], f32, kind="ExternalOutput")

    with tile.TileContext(nc) as tc:
        with ExitStack() as ctx:
            consts = ctx.enter_context(tc.tile_pool(name="consts", bufs=1))
            hs_pool = ctx.enter_context(tc.tile_pool(name="hsp", bufs=4))
            work = ctx.enter_context(tc.tile_pool(name="work", bufs=1))

            # Constants go on the scalar-engine HWDGE ring so they never
            # head-of-line-block the hidden_states stream on the sync ring.
            def cload(dram, shape, dt=f32):
                t = consts.tile(shape, dt, tag=dram.name)
                nc.scalar.dma_start(t[:], dram.ap())
                return t

            id_sb = cload(ident, [BL, BL])
            wro_sb = cload(wro, [128, NH * 5])
            # Big constants are staggered into the pooling loop below so they
            # don't compete with the hidden_states stream at startup.
            coeff_sb = consts.tile([128, L * 2 * BL * BL], fmm, tag="coeff")
            wd_sb = consts.tile([128, NH * BL * K], fmm, tag="wd")
            wuro_sb = consts.tile([K, BL * 5], f32, tag="wuro")
            bd_sb = consts.tile([BL, K], f32, tag="bd")
            bro_sb = consts.tile([BL, 5], f32, tag="bro")
            dmask64_sb = consts.tile([BL, BL * K], f32, tag="dmask64")
            dmask5_sb = consts.tile([BL, BL * 5], f32, tag="dmask5")

            # All remaining constants go on the SYNC ring, interleaved with the
            # hs chunks: the sync HWDGE ring is a single FIFO queue, so this
            # staggers them temporally instead of competing at startup.
            def load_coeff_blocks(ll):
                for c in range(2):
                    j = ll * 2 + c
                    blk = slice(j * BL * BL, (j + 1) * BL * BL)
                    nc.sync.dma_start(coeff_sb[:, blk], coeff.ap()[:, blk])

            def load_wd_chunk(r):
                blk = slice(r * BL * K, (r + 1) * BL * K)
                nc.sync.dma_start(wd_sb[:, blk], wd.ap()[:, blk])

            load_coeff_blocks(0)

            # ---- PE warm-up: ~4us of tiny matmuls so HAM reaches K=8/8
            # before the real stream starts (runs during startup DMAs) ----
            with tc.tile_pool(name="pwarm", bufs=1, space="PSUM") as pwarm:
                wps = pwarm.tile([BL, BL], f32)
                for _ in range(48):
                    nc.tensor.matmul(wps[:], id_sb[:], id_sb[:],
                                     start=True, stop=True)

            # ---- pooling: feats[b, h] = sum_{l,s} w_l * c[b,s] * hs[l,b,s,h] ----
            # Zero-embedded coeff columns route sample b's contribution to psum
            # row b, so feats accumulates directly as [BL, H] with b on
            # partitions.
            feats_sb = work.tile([BL, H], f32)
            with tc.tile_pool(name="pf", bufs=1, space="PSUM") as pf_pool:
                pf = pf_pool.tile([BL, H], f32)
                hs_ap = hs.ap()
                idx = 0
                n_idx = L * BL * 2
                for l in range(L):
                    for bp in range(BL // BPAIR):
                        # Interleave late-needed constants into the sync-ring
                        # FIFO, one layer (~110us) ahead of their use.
                        if bp == 2 and l + 1 < L:
                            load_coeff_blocks(l + 1)
                        if l >= 2 and bp in (4, 8, 12, 15):
                            load_wd_chunk((l - 2) * 4 + (4, 8, 12, 15).index(bp))
                        if l == 2 and bp == 6:
                            nc.sync.dma_start(wuro_sb[:], wuro.ap())
                            nc.sync.dma_start(bd_sb[:], bd.ap())
                            nc.sync.dma_start(bro_sb[:], bro.ap())
                        if l == 3 and bp == 6:
                            nc.sync.dma_start(dmask64_sb[:], dmask64.ap())
                            nc.sync.dma_start(dmask5_sb[:], dmask5.ap())
                        t = hs_pool.tile([128, BPAIR * 2 * 1024], fmm, tag="hst")
                        src = hs_ap[l, bp * BPAIR:(bp + 1) * BPAIR].rearrange(
                            "b (c p) h -> p b c h", p=128
                        )
                        dst = t[:].rearrange("p (b c h) -> p b c h", b=BPAIR, c=2)
                        nc.sync.dma_start(dst, src)
                        for bb in range(BPAIR):
                            b = bp * BPAIR + bb
                            for c in range(2):
                                j = l * 2 + c
                                lhs = coeff_sb[
                                    :, (j * BL + b) * BL:(j * BL + b + 1) * BL]
                                for hh in range(2):
                                    base = bb * 2048 + c * 1024 + hh * 512
                                    nc.tensor.matmul(
                                        pf[:, hh * 512:(hh + 1) * 512],
                                        lhs,
                                        t[:, base:base + 512],
                                        start=(idx == 0),
                                        stop=(idx == n_idx - 1),
                                    )
                                idx += 1
                nc.vector.tensor_copy(feats_sb[:], pf[:])
            # pf's 2 PSUM banks are free again here.

            from concourse import mybir as _mb
            with tc.tile_pool(name="pt", bufs=2, space="PSUM") as pt_pool, \
                 tc.tile_pool(name="pd", bufs=1, space="PSUM") as pd_pool, \
                 tc.tile_pool(name="ph", bufs=1, space="PSUM") as ph_pool:
                # ---- transpose feats to h-on-partitions ----
                # featsT[:, r*BL + b][p] = feats[b, 8p + r]
                featsT = work.tile([128, NH * BL], fmm)
                fview = feats_sb[:].rearrange("p (q r) -> p r q", r=NH)
                for r in range(NH):
                    pt = pt_pool.tile([128, BL], f32, tag="pt")
                    nc.tensor.transpose(pt[:], fview[:, r], id_sb[:])
                    nc.vector.tensor_copy(featsT[:, r * BL:(r + 1) * BL], pt[:])

                # ---- batched down-projection ----
                # bigD[m, b*K+k] = sum_h feats[m,h] * W_down[sec[b],h,k];
                # the diagonal m==b is the wanted z[b, k].
                bigD = pd_pool.tile([BL, BL * K], f32, tag="bigD")
                nmm = BL * K // 512                      # 4 matmuls of N=512
                for r in range(NH):
                    lhsT = featsT[:, r * BL:(r + 1) * BL]
                    for n in range(nmm):
                        nc.tensor.matmul(
                            bigD[:, n * 512:(n + 1) * 512],
                            lhsT,
                            wd_sb[:, r * BL * K + n * 512:
                                  r * BL * K + (n + 1) * 512],
                            start=(r == 0),
                            stop=(r == NH - 1),
                        )
                zmask = work.tile([BL, BL * K], f32)
                nc.vector.tensor_mul(zmask[:], bigD[:], dmask64_sb[:])
                z_bt = work.tile([BL, K], f32)
                nc.vector.tensor_reduce(
                    z_bt[:],
                    zmask[:].rearrange("p (g j) -> p j g", j=K),
                    _mb.AxisListType.X,
                    _mb.AluOpType.add,
                )
                zb = work.tile([BL, K], f32)
                nc.vector.tensor_add(zb[:], z_bt[:], bd_sb[:])
                h1_bt = work.tile([BL, K], f32)
                nc.scalar.activation(
                    h1_bt[:], zb[:], getattr(_mb.ActivationFunctionType, act_fn))
                # h1 with k on partitions for the head-A matmul
                pth = pt_pool.tile([K, BL], f32, tag="pt")
                nc.tensor.transpose(pth[:], h1_bt[:], id_sb[:])
                h1_sb = work.tile([K, BL], f32)
                nc.vector.tensor_copy(h1_sb[:], pth[:])

                # ---- heads ----
                # B-part: feats @ [W_reg|W_ord] -> pB[b, j]
                pB = ph_pool.tile([BL, 5], f32, tag="pB")
                for r in range(NH):
                    # head-B stays fp32 (fp32r rejects tiny moving dims)
                    nc.tensor.matmul(
                        pB[:],
                        featsT[:, r * BL:(r + 1) * BL].bitcast(f32),
                        wro_sb[:, r * 5:(r + 1) * 5],
                        start=(r == 0),
                        stop=(r == NH - 1),
                    )
                # A-part: h1[b] @ (W_up[sec[b]] @ W_ro) for all (m, b) pairs,
                # then diagonal-extract.
                pA = ph_pool.tile([BL, BL * 5], f32, tag="pA")
                nc.tensor.matmul(pA[:], h1_sb[:], wuro_sb[:], start=True, stop=True)
                tmpA = work.tile([BL, BL * 5], f32)
                nc.vector.tensor_mul(tmpA[:], pA[:], dmask5_sb[:])
                redA = work.tile([BL, 5], f32)
                nc.vector.tensor_reduce(
                    redA[:],
                    tmpA[:].rearrange("p (g j) -> p j g", j=5),
                    _mb.AxisListType.X,
                    _mb.AluOpType.add,
                )
                o1 = work.tile([BL, 5], f32)
                nc.vector.tensor_add(o1[:], pB[:], redA[:])
                o2 = work.tile([BL, 5], f32)
                nc.vector.tensor_add(o2[:], o1[:], bro_sb[:])
                nc.sync.dma_start(out.ap(), o2[:])

    nc.compile()
    return nc


def _softmax(x):
    e = np.exp(x - x.max())
    return e / e.sum()


def _prepare_inputs(hidden_states, attention_mask, section_id, layer_weights,
                    W_down, b_down, W_up, b_up, W_reg, b_reg, W_ord, b_ord):
    hidden_states = np.asarray(hidden_states, dtype=np.float32)
    mask = np.asarray(attention_mask)
    sec = np.asarray(section_id).astype(np.int64)
    lw = np.asarray(layer_weights, dtype=np.float32)
    W_down = np.asarray(W_down, dtype=np.float32)
    b_down = np.asarray(b_down, dtype=np.float32)
    W_up = np.asarray(W_up, dtype=np.float32)
    b_up = np.asarray(b_up, dtype=np.float32)
    W_reg = np.asarray(W_reg, dtype=np.float32)
    b_reg = np.asarray(b_reg, dtype=np.float32)
    W_ord = np.asarray(W_ord, dtype=np.float32)
    b_ord = np.asarray(b_ord, dtype=np.float32)

    w = _softmax(lw)                                     # [L]
    mf = mask.astype(np.float32)                         # [B, S]
    msum = np.maximum(mf.sum(axis=1), 1e-6)              # [B]
    cmask = mf / msum[:, None]                           # [B, S]
    base = cmask.reshape(B, 2, 128).transpose(2, 1, 0)   # [p, c, b]
    coeff_all = w[None, :, None, None] * base[:, None, :, :]   # [p, l, c, b]

    wro_np = np.concatenate([W_reg, W_ord], axis=1)      # [H, 5]
    wro_dev = np.ascontiguousarray(wro_np.reshape(128, NH * 5))
    wd_all = W_down[sec]                                 # [B, H, K]
    # [p, r, b, k] with h = 8p + r
    wd_dev_all = wd_all.reshape(B, 128, NH, K).transpose(1, 2, 0, 3)
    wu_ro = np.einsum("ekh,hj->ekj", W_up, wro_np)       # [NSEC, K, 5]
    wuro_all = wu_ro[sec].transpose(1, 0, 2)             # [K, B, 5]
    bd_all = b_down[sec]                                 # [B, K]
    bro_all = b_up[sec] @ wro_np + np.concatenate([b_reg, b_ord])[None]  # [B, 5]

    dmask64_np = np.zeros((BL, BL * K), np.float32)
    dmask5_np = np.zeros((BL, BL * 5), np.float32)
    for i in range(BL):
        dmask64_np[i, i * K:(i + 1) * K] = 1.0
        dmask5_np[i, i * 5:(i + 1) * 5] = 1.0
    ident = np.eye(BL, dtype=np.float32)

    in_maps = []
    idx = np.arange(BL)
    for core in range(NCORES):
        sl = slice(core * BL, (core + 1) * BL)
        cc = np.zeros((128, L, 2, BL, BL), np.float32)
        cc[:, :, :, idx, idx] = coeff_all[:, :, :, sl]
        in_maps.append({
            "hs": np.ascontiguousarray(hidden_states[:, sl]),
            "coeff": np.ascontiguousarray(cc.reshape(128, L * 2 * BL * BL)),
            "wd": np.ascontiguousarray(
                wd_dev_all[:, :, sl].reshape(128, NH * BL * K)),
            "wro": wro_dev,
            "wuro": np.ascontiguousarray(wuro_all[:, sl].reshape(K, BL * 5)),
            "dmask64": dmask64_np,
            "dmask5": dmask5_np,
            "bd": np.ascontiguousarray(bd_all[sl]),
            "bro": np.ascontiguousarray(bro_all[sl]),
            "ident": ident,
        })
    return in_maps


def get_module(act_fn="Gelu"):
    key = "nc_" + act_fn
    if key not in _CACHE:
        _CACHE[key] = _build_module(act_fn)
    return _CACHE[key]


def kernel(hidden_states, attention_mask, section_id, layer_weights,
           W_down, b_down, W_up, b_up, W_reg, b_reg, W_ord, b_ord):
    global LAST_RESULT
    from concourse.bass_utils import run_bass_kernel_spmd

    in_maps = _prepare_inputs(
        hidden_states, attention_mask, section_id, layer_weights,
        W_down, b_down, W_up, b_up, W_reg, b_reg, W_ord, b_ord)
    nc = get_module()
    res = run_bass_kernel_spmd(nc, in_maps, list(range(NCORES)))
    LAST_RESULT = res
    out = np.concatenate([res.results[c]["out"] for c in range(NCORES)], axis=0)
    reg = np.ascontiguousarray(out[:, 0])
    ord_logits = np.ascontiguousarray(out[:, 1:5])
    return reg, ord_logits
